# revision 6
# baseline (speedup 1.0000x reference)
"""CourierEncoder fused kernel for 8 Trainium2 NeuronCores.

Data-parallel over the batch: each core processes B/8 = 32768 rows.
Per 512-row tile (all matmuls bf16 -> fp32 PSUM):
  embeds:  K=1 outer-product matmuls (w (x) coord), cos folded as Sin(z+pi/2),
           ACT Sin / Lrelu with exact per-partition f32 biases
  layer 1: feature-major, 6 matmuls [128,128]@[128,512]
  layer 2: batch-major (lhsT = h1T slices), bias b2 via ones (x) b2 matmul,
           LeakyReLU on DVE via scalar_tensor_tensor (max(0.01*z, z))
"""

import math

import numpy as np
import ml_dtypes

import concourse.bass as bass
import concourse.tile as tile
import concourse.mybir as mybir
from concourse import bacc
from concourse.bass_utils import run_bass_kernel_spmd

B = 262144
NCORES = 8
R = B // NCORES          # rows per core
TILE = 512               # rows per tile
NT = R // TILE           # tiles per core
PED = 256
NED = 128
CED = 256
Q = PED // 4             # 64
ALPHA = 0.01

F32 = mybir.dt.float32
BF16 = mybir.dt.bfloat16
AF = mybir.ActivationFunctionType
ALU = mybir.AluOpType

_CACHE = {}


def _build():
    nc = bacc.Bacc()
    xy = nc.dram_tensor("xy", [R, 2], F32, kind="ExternalInput")
    t = nc.dram_tensor("t", [R, 1], F32, kind="ExternalInput")
    emb_w = nc.dram_tensor("emb_w", [3, 128], BF16, kind="ExternalInput")
    biases = nc.dram_tensor("biases", [128, 5], F32, kind="ExternalInput")
    w1p = nc.dram_tensor("w1p", [128, 3, 2, 128], BF16, kind="ExternalInput")
    w2p = nc.dram_tensor("w2p", [128, 2, 256], BF16, kind="ExternalInput")
    b2rep = nc.dram_tensor("b2rep", [2, 512], BF16, kind="ExternalInput")
    out = nc.dram_tensor("out", [R, 256], F32, kind="ExternalOutput")

    with tile.TileContext(nc) as tc:
        with (
            tc.tile_pool(name="const", bufs=1) as const,
            tc.tile_pool(name="io", bufs=4) as io,
            tc.tile_pool(name="acts", bufs=2) as acts,
            tc.tile_pool(name="outp", bufs=3) as outp,
            tc.tile_pool(name="ps_emb", bufs=3, space="PSUM") as ps_emb,
            tc.tile_pool(name="ps_l1", bufs=2, space="PSUM") as ps_l1,
            tc.tile_pool(name="ps_l2", bufs=2, space="PSUM") as ps_l2,
        ):
            emb_w_sb = const.tile([65, 128], BF16)
            bias_sb = const.tile([128, 5], F32)
            w1_sb = const.tile([128, 3, 2, 128], BF16)
            w2_sb = const.tile([128, 2, 256], BF16)
            b2_sb = const.tile([2, 512], BF16)
            ones_sb = const.tile([2, 128], BF16)
            for c in range(3):
                nc.sync.dma_start(out=emb_w_sb[32 * c:32 * c + 1, :],
                                  in_=emb_w[c:c + 1, :])
            nc.sync.dma_start(out=bias_sb, in_=biases[:, :])
            nc.sync.dma_start(out=w1_sb, in_=w1p[:, :, :, :])
            nc.sync.dma_start(out=w2_sb, in_=w2p[:, :, :])
            nc.sync.dma_start(out=b2_sb, in_=b2rep[:, :])
            nc.vector.memset(ones_sb, 1.0)

            for it in range(NT):
                base = it * TILE
                # -- load + cast coords -------------------------------------
                xyt_f = io.tile([65, TILE], F32)
                xyt_b = io.tile([65, TILE], BF16)
                nc.sync.dma_start(
                    out=xyt_f[0:1, :],
                    in_=xy[base:base + TILE, 0:1].rearrange("n c -> c n"),
                )
                nc.sync.dma_start(
                    out=xyt_f[32:33, :],
                    in_=xy[base:base + TILE, 1:2].rearrange("n c -> c n"),
                )
                nc.sync.dma_start(
                    out=xyt_f[64:65, :],
                    in_=t[base:base + TILE, :].rearrange("n c -> c n"),
                )
                nc.vector.tensor_copy(out=xyt_b, in_=xyt_f)

                # -- embeddings (outer products) ----------------------------
                hT = acts.tile([128, 3, TILE], BF16)
                for c in range(3):
                    emb_ps = ps_emb.tile([128, TILE], F32, tag="emb")
                    nc.tensor.matmul(
                        emb_ps,
                        emb_w_sb[32 * c:32 * c + 1, :],
                        xyt_b[32 * c:32 * c + 1, :],
                        start=True, stop=True,
                    )
                    nc.scalar.activation(
                        out=hT[:, c, :], in_=emb_ps,
                        func=(AF.Sin if c < 2 else AF.Prelu),
                        bias=bias_sb[:, c:c + 1], alpha=ALPHA)

                # -- layer 1 (feature-major) --------------------------------
                h1T = acts.tile([128, 2, TILE], BF16)
                for mc in range(2):
                    l1_ps = ps_l1.tile([128, TILE], F32, tag="l1")
                    for kc in range(3):
                        nc.tensor.matmul(
                            l1_ps,
                            w1_sb[:, kc, mc, :],
                            hT[:, kc, :],
                            start=(kc == 0), stop=(kc == 2),
                        )
                    nc.scalar.activation(out=h1T[:, mc, :], in_=l1_ps,
                                         func=AF.Prelu, bias=bias_sb[:, 3 + mc:4 + mc],
                                         alpha=ALPHA)

                # -- layer 2 (batch-major) + LeakyReLU + store --------------
                o_sb = outp.tile([128, 4, 256], F32)
                for half in range(2):
                    l2_ps = ps_l2.tile([128, 2, 256], F32, tag="l2")
                    nc.tensor.matmul(
                        l2_ps,
                        ones_sb[:, :],
                        b2_sb[:, :],
                        start=True, stop=False,
                        skip_group_check=True,
                    )
                    for rr in range(2):
                        r = 2 * half + rr
                        for kc in range(2):
                            nc.tensor.matmul(
                                l2_ps[:, rr, :],
                                h1T[:, kc, r * 128:(r + 1) * 128],
                                w2_sb[:, kc, :],
                                start=False, stop=(kc == 1),
                                skip_group_check=True,
                            )
                    tmp_sb = outp.tile([128, 2, 256], F32, tag="l2tmp")
                    nc.vector.tensor_scalar_mul(out=tmp_sb, in0=l2_ps, scalar1=ALPHA)
                    nc.vector.tensor_max(out=o_sb[:, 2 * half:2 * half + 2, :],
                                         in0=l2_ps, in1=tmp_sb)
                nc.sync.dma_start(
                    out=out[base:base + TILE, :].rearrange("(r p) m -> p r m", p=128),
                    in_=o_sb,
                )
    nc.finalize()
    return nc


def _prep_weights(inputs):
    f = {k: np.asarray(v, dtype=np.float32) for k, v in inputs.items()}
    bf = ml_dtypes.bfloat16

    emb_w = np.stack([
        np.concatenate([f["w_sx"].ravel(), f["w_cx"].ravel()]),
        np.concatenate([f["w_sy"].ravel(), f["w_cy"].ravel()]),
        f["w_t"].ravel(),
    ]).astype(bf)

    biases = np.zeros((128, 5), np.float32)
    biases[:, 0] = np.concatenate([f["b_sx"], f["b_cx"] + math.pi / 2])
    biases[:, 1] = np.concatenate([f["b_sy"], f["b_cy"] + math.pi / 2])
    biases[:, 2] = f["b_t"]
    biases[:, 3] = f["b1"][0:128]
    biases[:, 4] = f["b1"][128:256]

    w1p = f["w1"].reshape(3, 128, 2, 128).transpose(1, 0, 2, 3).astype(bf)
    w2p = f["w2"].reshape(2, 128, 256).transpose(1, 0, 2).astype(bf)

    b2 = f["b2"]
    b2_hi = b2.astype(bf).astype(np.float32)
    b2_lo = b2 - b2_hi
    b2rep = np.stack([
        np.concatenate([b2_hi, b2_hi]),
        np.concatenate([b2_lo, b2_lo]),
    ]).astype(bf)

    return {
        "emb_w": emb_w,
        "biases": biases,
        "w1p": np.ascontiguousarray(w1p),
        "w2p": np.ascontiguousarray(w2p),
        "b2rep": b2rep,
    }


def kernel(**inputs):
    if "nc" not in _CACHE:
        _CACHE["nc"] = _build()
    nc = _CACHE["nc"]

    w = _prep_weights(inputs)
    xy = np.ascontiguousarray(np.asarray(inputs["xy"], dtype=np.float32))
    t = np.ascontiguousarray(np.asarray(inputs["t"], dtype=np.float32))

    in_maps = []
    for c in range(NCORES):
        lo, hi = c * R, (c + 1) * R
        in_maps.append({
            "xy": xy[lo:hi], "t": t[lo:hi], **w,
        })

    res = run_bass_kernel_spmd(nc, in_maps, core_ids=list(range(NCORES)))
    _CACHE["last_res"] = res
    return np.concatenate([res.results[c]["out"] for c in range(NCORES)], axis=0)


# revision 8
# speedup vs baseline: 1.0171x; 1.0171x over previous
"""CourierEncoder fused kernel for 8 Trainium2 NeuronCores.

Data-parallel over the batch: each core processes B/8 = 32768 rows.
Per 512-row tile (all matmuls bf16 -> fp32 PSUM):
  embeds:  K=1 outer-product matmuls (w (x) coord), cos folded as Sin(z+pi/2),
           ACT Sin / Lrelu with exact per-partition f32 biases
  layer 1: feature-major, 6 matmuls [128,128]@[128,512]
  layer 2: batch-major (lhsT = h1T slices), bias b2 via ones (x) b2 matmul,
           LeakyReLU on DVE via scalar_tensor_tensor (max(0.01*z, z))
"""

import math

import numpy as np
import ml_dtypes

import concourse.bass as bass
import concourse.tile as tile
import concourse.mybir as mybir
from concourse import bacc
from concourse.bass_utils import run_bass_kernel_spmd

B = 262144
NCORES = 8
R = B // NCORES          # rows per core
TILE = 512               # rows per tile
NT = R // TILE           # tiles per core
PED = 256
NED = 128
CED = 256
Q = PED // 4             # 64
ALPHA = 0.01

F32 = mybir.dt.float32
BF16 = mybir.dt.bfloat16
AF = mybir.ActivationFunctionType
ALU = mybir.AluOpType

_CACHE = {}


def _build():
    nc = bacc.Bacc()
    xy = nc.dram_tensor("xy", [R, 2], F32, kind="ExternalInput")
    t = nc.dram_tensor("t", [R, 1], F32, kind="ExternalInput")
    emb_w = nc.dram_tensor("emb_w", [3, 128], BF16, kind="ExternalInput")
    biases = nc.dram_tensor("biases", [128, 5], F32, kind="ExternalInput")
    w1p = nc.dram_tensor("w1p", [128, 3, 2, 128], BF16, kind="ExternalInput")
    w2p = nc.dram_tensor("w2p", [128, 2, 256], BF16, kind="ExternalInput")
    b2rep = nc.dram_tensor("b2rep", [2, 512], BF16, kind="ExternalInput")
    out = nc.dram_tensor("out", [R, 256], F32, kind="ExternalOutput")

    with tile.TileContext(nc) as tc:
        with (
            tc.tile_pool(name="const", bufs=1) as const,
            tc.tile_pool(name="io", bufs=4) as io,
            tc.tile_pool(name="acts", bufs=2) as acts,
            tc.tile_pool(name="outp", bufs=3) as outp,
            tc.tile_pool(name="ps_emb", bufs=3, space="PSUM") as ps_emb,
            tc.tile_pool(name="ps_l1", bufs=2, space="PSUM") as ps_l1,
            tc.tile_pool(name="ps_l2", bufs=1, space="PSUM") as ps_l2,
        ):
            emb_w_sb = const.tile([65, 128], BF16)
            bias_sb = const.tile([128, 5], F32)
            w1_sb = const.tile([128, 3, 2, 128], BF16)
            w2_sb = const.tile([128, 2, 256], BF16)
            b2_sb = const.tile([2, 512], BF16)
            ones_sb = const.tile([2, 128], BF16)
            for c in range(3):
                nc.sync.dma_start(out=emb_w_sb[32 * c:32 * c + 1, :],
                                  in_=emb_w[c:c + 1, :])
            nc.sync.dma_start(out=bias_sb, in_=biases[:, :])
            nc.sync.dma_start(out=w1_sb, in_=w1p[:, :, :, :])
            nc.sync.dma_start(out=w2_sb, in_=w2p[:, :, :])
            nc.sync.dma_start(out=b2_sb, in_=b2rep[:, :])
            nc.vector.memset(ones_sb, 1.0)

            for it in range(NT):
                base = it * TILE
                # -- load + cast coords -------------------------------------
                xyt_f = io.tile([65, TILE], F32)
                xyt_b = io.tile([65, TILE], BF16)
                nc.sync.dma_start(
                    out=xyt_f[0:1, :],
                    in_=xy[base:base + TILE, 0:1].rearrange("n c -> c n"),
                )
                nc.sync.dma_start(
                    out=xyt_f[32:33, :],
                    in_=xy[base:base + TILE, 1:2].rearrange("n c -> c n"),
                )
                nc.sync.dma_start(
                    out=xyt_f[64:65, :],
                    in_=t[base:base + TILE, :].rearrange("n c -> c n"),
                )
                nc.vector.tensor_copy(out=xyt_b, in_=xyt_f)

                # -- embeddings (outer products) ----------------------------
                hT = [acts.tile([128, TILE], BF16, tag=f"hT{c}", name=f"hT{c}") for c in range(3)]
                for c in range(3):
                    emb_ps = ps_emb.tile([128, TILE], F32, tag="emb")
                    nc.tensor.matmul(
                        emb_ps,
                        emb_w_sb[32 * c:32 * c + 1, :],
                        xyt_b[32 * c:32 * c + 1, :],
                        start=True, stop=True,
                    )
                    nc.scalar.activation(
                        out=hT[c], in_=emb_ps,
                        func=(AF.Sin if c < 2 else AF.Prelu),
                        bias=bias_sb[:, c:c + 1], alpha=ALPHA)

                # -- layer 1 (feature-major) --------------------------------
                h1T = [acts.tile([128, TILE], BF16, tag=f"h1T{mc}", name=f"h1T{mc}") for mc in range(2)]
                for mc in range(2):
                    l1_ps = ps_l1.tile([128, TILE], F32, tag="l1")
                    for kc in range(3):
                        nc.tensor.matmul(
                            l1_ps,
                            w1_sb[:, kc, mc, :],
                            hT[kc],
                            start=(kc == 0), stop=(kc == 2),
                        )
                    nc.scalar.activation(out=h1T[mc], in_=l1_ps,
                                         func=AF.Prelu, bias=bias_sb[:, 3 + mc:4 + mc],
                                         alpha=ALPHA)

                # -- layer 2 (batch-major) + LeakyReLU + store --------------
                o_sb = outp.tile([128, 4, 256], F32)
                l2_ps = ps_l2.tile([128, 4, 256], F32, tag="l2")
                for half in range(2):
                    nc.tensor.matmul(
                        l2_ps[:, 2 * half:2 * half + 2, :],
                        ones_sb[:, :],
                        b2_sb[:, :],
                        start=True, stop=False,
                        skip_group_check=True,
                    )
                for r in range(4):
                    for kc in range(2):
                        nc.tensor.matmul(
                            l2_ps[:, r, :],
                            h1T[kc][:, r * 128:(r + 1) * 128],
                            w2_sb[:, kc, :],
                            start=False, stop=(kc == 1),
                            skip_group_check=True,
                        )
                tmp_sb = outp.tile([128, 4, 256], F32, tag="l2tmp")
                nc.vector.tensor_scalar_mul(out=tmp_sb, in0=l2_ps, scalar1=ALPHA)
                nc.vector.tensor_max(out=o_sb, in0=l2_ps, in1=tmp_sb)
                nc.sync.dma_start(
                    out=out[base:base + TILE, :].rearrange("(r p) m -> p r m", p=128),
                    in_=o_sb,
                )
    nc.finalize()
    return nc


def _prep_weights(inputs):
    f = {k: np.asarray(v, dtype=np.float32) for k, v in inputs.items()}
    bf = ml_dtypes.bfloat16

    emb_w = np.stack([
        np.concatenate([f["w_sx"].ravel(), f["w_cx"].ravel()]),
        np.concatenate([f["w_sy"].ravel(), f["w_cy"].ravel()]),
        f["w_t"].ravel(),
    ]).astype(bf)

    biases = np.zeros((128, 5), np.float32)
    biases[:, 0] = np.concatenate([f["b_sx"], f["b_cx"] + math.pi / 2])
    biases[:, 1] = np.concatenate([f["b_sy"], f["b_cy"] + math.pi / 2])
    biases[:, 2] = f["b_t"]
    biases[:, 3] = f["b1"][0:128]
    biases[:, 4] = f["b1"][128:256]

    w1p = f["w1"].reshape(3, 128, 2, 128).transpose(1, 0, 2, 3).astype(bf)
    w2p = f["w2"].reshape(2, 128, 256).transpose(1, 0, 2).astype(bf)

    b2 = f["b2"]
    b2_hi = b2.astype(bf).astype(np.float32)
    b2_lo = b2 - b2_hi
    b2rep = np.stack([
        np.concatenate([b2_hi, b2_hi]),
        np.concatenate([b2_lo, b2_lo]),
    ]).astype(bf)

    return {
        "emb_w": emb_w,
        "biases": biases,
        "w1p": np.ascontiguousarray(w1p),
        "w2p": np.ascontiguousarray(w2p),
        "b2rep": b2rep,
    }


def kernel(**inputs):
    if "nc" not in _CACHE:
        _CACHE["nc"] = _build()
    nc = _CACHE["nc"]

    w = _prep_weights(inputs)
    xy = np.ascontiguousarray(np.asarray(inputs["xy"], dtype=np.float32))
    t = np.ascontiguousarray(np.asarray(inputs["t"], dtype=np.float32))

    in_maps = []
    for c in range(NCORES):
        lo, hi = c * R, (c + 1) * R
        in_maps.append({
            "xy": xy[lo:hi], "t": t[lo:hi], **w,
        })

    res = run_bass_kernel_spmd(nc, in_maps, core_ids=list(range(NCORES)))
    _CACHE["last_res"] = res
    return np.concatenate([res.results[c]["out"] for c in range(NCORES)], axis=0)


# revision 9
# speedup vs baseline: 1.0868x; 1.0685x over previous
"""CourierEncoder fused kernel for 8 Trainium2 NeuronCores.

Data-parallel over the batch: each core processes B/8 = 32768 rows.
Per 512-row tile (all matmuls bf16 -> fp32 PSUM):
  embeds:  K=1 outer-product matmuls (w (x) coord), cos folded as Sin(z+pi/2),
           ACT Sin / Lrelu with exact per-partition f32 biases
  layer 1: feature-major, 6 matmuls [128,128]@[128,512]
  layer 2: batch-major (lhsT = h1T slices), bias b2 via ones (x) b2 matmul,
           LeakyReLU on DVE via scalar_tensor_tensor (max(0.01*z, z))
"""

import math

import numpy as np
import ml_dtypes

import concourse.bass as bass
import concourse.tile as tile
import concourse.mybir as mybir
from concourse import bacc
from concourse.bass_utils import run_bass_kernel_spmd

B = 262144
NCORES = 8
R = B // NCORES          # rows per core
TILE = 512               # rows per tile
NT = R // TILE           # tiles per core
PED = 256
NED = 128
CED = 256
Q = PED // 4             # 64
ALPHA = 0.01

F32 = mybir.dt.float32
BF16 = mybir.dt.bfloat16
AF = mybir.ActivationFunctionType
ALU = mybir.AluOpType

_CACHE = {}


def _build():
    nc = bacc.Bacc()
    xy = nc.dram_tensor("xy", [R, 2], F32, kind="ExternalInput")
    t = nc.dram_tensor("t", [R, 1], F32, kind="ExternalInput")
    emb_w = nc.dram_tensor("emb_w", [3, 128], BF16, kind="ExternalInput")
    biases = nc.dram_tensor("biases", [128, 5], F32, kind="ExternalInput")
    w1p = nc.dram_tensor("w1p", [128, 3, 2, 128], BF16, kind="ExternalInput")
    w2p = nc.dram_tensor("w2p", [128, 2, 256], BF16, kind="ExternalInput")
    b2rep = nc.dram_tensor("b2rep", [2, 512], BF16, kind="ExternalInput")
    out = nc.dram_tensor("out", [R, 256], F32, kind="ExternalOutput")

    with tile.TileContext(nc) as tc:
        with (
            tc.tile_pool(name="const", bufs=1) as const,
            tc.tile_pool(name="io", bufs=4) as io,
            tc.tile_pool(name="acts", bufs=3) as acts,
            tc.tile_pool(name="outp", bufs=4) as outp,
            tc.tile_pool(name="ps_emb", bufs=1, space="PSUM") as ps_emb,
            tc.tile_pool(name="ps_l1", bufs=1, space="PSUM") as ps_l1,
            tc.tile_pool(name="ps_l2", bufs=1, space="PSUM") as ps_l2,
        ):
            emb_w_sb = const.tile([65, 128], BF16)
            bias_sb = const.tile([128, 5], F32)
            w1_sb = const.tile([128, 3, 2, 128], BF16)
            w2_sb = const.tile([128, 2, 256], BF16)
            b2_sb = const.tile([2, 512], BF16)
            ones_sb = const.tile([2, 128], BF16)
            for c in range(3):
                nc.sync.dma_start(out=emb_w_sb[32 * c:32 * c + 1, :],
                                  in_=emb_w[c:c + 1, :])
            nc.sync.dma_start(out=bias_sb, in_=biases[:, :])
            nc.sync.dma_start(out=w1_sb, in_=w1p[:, :, :, :])
            nc.sync.dma_start(out=w2_sb, in_=w2p[:, :, :])
            nc.sync.dma_start(out=b2_sb, in_=b2rep[:, :])
            nc.vector.memset(ones_sb, 1.0)

            for it in range(NT):
                base = it * TILE
                # -- load + cast coords -------------------------------------
                xyt_f = io.tile([65, TILE], F32)
                xyt_b = io.tile([65, TILE], BF16)
                nc.sync.dma_start(
                    out=xyt_f[0:1, :],
                    in_=xy[base:base + TILE, 0:1].rearrange("n c -> c n"),
                )
                nc.sync.dma_start(
                    out=xyt_f[32:33, :],
                    in_=xy[base:base + TILE, 1:2].rearrange("n c -> c n"),
                )
                nc.sync.dma_start(
                    out=xyt_f[64:65, :],
                    in_=t[base:base + TILE, :].rearrange("n c -> c n"),
                )
                nc.vector.tensor_copy(out=xyt_b, in_=xyt_f)

                # -- embeddings (outer products) ----------------------------
                emb_ps = ps_emb.tile([128, 3, TILE], F32)
                for c in range(3):
                    nc.tensor.matmul(
                        emb_ps[:, c, :],
                        emb_w_sb[32 * c:32 * c + 1, :],
                        xyt_b[32 * c:32 * c + 1, :],
                        start=True, stop=True,
                    )
                hT = acts.tile([128, 3, TILE], BF16)
                nc.scalar.activation(out=hT[:, 0, :], in_=emb_ps[:, 0, :],
                                     func=AF.Sin, bias=bias_sb[:, 0:1])
                nc.scalar.activation(out=hT[:, 1, :], in_=emb_ps[:, 1, :],
                                     func=AF.Sin, bias=bias_sb[:, 1:2])
                nc.scalar.activation(out=hT[:, 2, :], in_=emb_ps[:, 2, :],
                                     func=AF.Prelu, bias=bias_sb[:, 2:3],
                                     alpha=ALPHA)

                # -- layer 1 (feature-major) --------------------------------
                l1_ps = ps_l1.tile([128, 2, TILE], F32)
                for mc in range(2):
                    for kc in range(3):
                        nc.tensor.matmul(
                            l1_ps[:, mc, :],
                            w1_sb[:, kc, mc, :],
                            hT[:, kc, :],
                            start=(kc == 0), stop=(kc == 2),
                        )
                h1T = acts.tile([128, 2, TILE], BF16)
                for mc in range(2):
                    nc.scalar.activation(out=h1T[:, mc, :], in_=l1_ps[:, mc, :],
                                         func=AF.Prelu, bias=bias_sb[:, 3 + mc:4 + mc],
                                         alpha=ALPHA)

                # -- layer 2 (batch-major) + LeakyReLU + store --------------
                o_sb = outp.tile([128, 4, 256], F32)
                l2_ps = ps_l2.tile([128, 4, 256], F32, tag="l2")
                for half in range(2):
                    nc.tensor.matmul(
                        l2_ps[:, 2 * half:2 * half + 2, :],
                        ones_sb[:, :],
                        b2_sb[:, :],
                        start=True, stop=False,
                        skip_group_check=True,
                    )
                for r in range(4):
                    for kc in range(2):
                        nc.tensor.matmul(
                            l2_ps[:, r, :],
                            h1T[:, kc, r * 128:(r + 1) * 128],
                            w2_sb[:, kc, :],
                            start=False, stop=(kc == 1),
                            skip_group_check=True,
                        )
                tmp_sb = outp.tile([128, 4, 256], F32, tag="l2tmp")
                nc.vector.tensor_scalar_mul(out=tmp_sb, in0=l2_ps, scalar1=ALPHA)
                nc.vector.tensor_max(out=o_sb, in0=l2_ps, in1=tmp_sb)
                nc.sync.dma_start(
                    out=out[base:base + TILE, :].rearrange("(r p) m -> p r m", p=128),
                    in_=o_sb,
                )
    nc.finalize()
    return nc


def _prep_weights(inputs):
    f = {k: np.asarray(v, dtype=np.float32) for k, v in inputs.items()}
    bf = ml_dtypes.bfloat16

    emb_w = np.stack([
        np.concatenate([f["w_sx"].ravel(), f["w_cx"].ravel()]),
        np.concatenate([f["w_sy"].ravel(), f["w_cy"].ravel()]),
        f["w_t"].ravel(),
    ]).astype(bf)

    biases = np.zeros((128, 5), np.float32)
    biases[:, 0] = np.concatenate([f["b_sx"], f["b_cx"] + math.pi / 2])
    biases[:, 1] = np.concatenate([f["b_sy"], f["b_cy"] + math.pi / 2])
    biases[:, 2] = f["b_t"]
    biases[:, 3] = f["b1"][0:128]
    biases[:, 4] = f["b1"][128:256]

    w1p = f["w1"].reshape(3, 128, 2, 128).transpose(1, 0, 2, 3).astype(bf)
    w2p = f["w2"].reshape(2, 128, 256).transpose(1, 0, 2).astype(bf)

    b2 = f["b2"]
    b2_hi = b2.astype(bf).astype(np.float32)
    b2_lo = b2 - b2_hi
    b2rep = np.stack([
        np.concatenate([b2_hi, b2_hi]),
        np.concatenate([b2_lo, b2_lo]),
    ]).astype(bf)

    return {
        "emb_w": emb_w,
        "biases": biases,
        "w1p": np.ascontiguousarray(w1p),
        "w2p": np.ascontiguousarray(w2p),
        "b2rep": b2rep,
    }


def kernel(**inputs):
    if "nc" not in _CACHE:
        _CACHE["nc"] = _build()
    nc = _CACHE["nc"]

    w = _prep_weights(inputs)
    xy = np.ascontiguousarray(np.asarray(inputs["xy"], dtype=np.float32))
    t = np.ascontiguousarray(np.asarray(inputs["t"], dtype=np.float32))

    in_maps = []
    for c in range(NCORES):
        lo, hi = c * R, (c + 1) * R
        in_maps.append({
            "xy": xy[lo:hi], "t": t[lo:hi], **w,
        })

    res = run_bass_kernel_spmd(nc, in_maps, core_ids=list(range(NCORES)))
    _CACHE["last_res"] = res
    return np.concatenate([res.results[c]["out"] for c in range(NCORES)], axis=0)
